# revision 22
# baseline (speedup 1.0000x reference)
"""DeepseekOCR text MoE layer on 8 Trainium2 NeuronCores.

Expert-parallel: 4 routed experts per core (bucketed by token count so
every core's slot j has a similar load); shared expert tensor-sharded
over its intermediate dim (352 columns per core). Router + token
gather/scatter run on host (full-I/O contract); all matmuls run on
device in bf16 with fp32 PSUM accumulation; outputs stored bf16.

Device program per core, per expert cycle j:
  phase A (expert j):   hT[h,c] = silu(wg.T @ xgT) * (wu.T @ xgT)
  shared quarter q=j:   silu-mlp on a 512-token slice (this is the DMA
                        slack window for wd(j) + next expert's weights)
  phase B (expert j):   yT[d,c] = wd.T-tiles @ hT
Host: out = scatter_add(yT * combine_w) + sum_cores(ys).

Engine discipline: ALL loads issue on sync (emission ordered so a
semaphore-gated load never sits ahead of an earlier-needed one);
stores + silu on scalar (stores are gated just-in-time so they never
block a silu the PE pipeline needs). Per-slot dram tensors keep every
transfer contiguous per partition (fat descriptors).
"""

import numpy as np
import ml_dtypes

import concourse.bacc as bacc
import concourse.mybir as mybir
import concourse.tile as tile
from concourse.bass_utils import run_bass_kernel_spmd

B, S, D = 2, 1024, 2048
E, H, K = 32, 1408, 6
H_SHARED = 2816
ROUTED_SCALE = 1.0
T = B * S                      # 2048 tokens
N_CORES = 8
E_LOC = E // N_CORES           # 4 experts per core
HS_LOC = H_SHARED // N_CORES   # 352 shared-intermediate cols per core
HS_PAD = 384                   # padded to 3 k-tiles of 128
NH = H // 128                  # 11 h-tiles per routed expert
ND = D // 512                  # 4 d-groups (512 cols each)
NKD = D // 128                 # 16 contraction k-tiles over D
NSH = HS_PAD // 128            # 3 h-tiles for shared
TQ = 512                       # shared-expert token chunk
NTQ = T // TQ                  # 4 chunks
NXG = 4                        # xg split into k-chunks for early start
KC = NKD // NXG                # 4 k-tiles per xg chunk

BF16 = ml_dtypes.bfloat16
f32 = mybir.dt.float32
bf16 = mybir.dt.bfloat16

LAST_RESULTS = None            # BassKernelResults of the latest run (for test harness)


def _route(x, gate_w):
    """Greedy top-k softmax router, fp32 numpy (matches jax.lax.top_k order)."""
    logits = x @ gate_w.T                              # [T, E]
    m = logits.max(-1, keepdims=True)
    ex = np.exp(logits - m)
    scores = ex / ex.sum(-1, keepdims=True)
    topk_i = np.argsort(-scores, axis=-1, kind="stable")[:, :K]
    topk_w = np.take_along_axis(scores, topk_i, -1) * ROUTED_SCALE
    return topk_i, topk_w.astype(np.float32)


def _expert_mlp(nc, pools, slabs, C, hT_tag, nh):
    """Emit phase A (gate/up + silu*mul -> hT) for one expert."""
    psA, tmp_p, ht_p = pools
    xg_chunks, w_slabs = slabs                  # w_slabs[h] = (gate_slab, up_slab)
    NCC = -(-C // 512)
    hT = ht_p.tile([128, nh, C], bf16, tag=hT_tag)
    for h in range(nh):
        wg_s, wu_s = w_slabs[h]
        for cc in range(NCC):
            w = min(512, C - cc * 512)
            cs = slice(cc * 512, cc * 512 + w)
            pg = psA.tile([128, w], f32, tag="psA")
            for k in range(NKD):
                nc.tensor.matmul(pg[:], wg_s[:, k], xg_chunks[k // KC][:, k % KC, cs],
                                 start=(k == 0), stop=(k == NKD - 1))
            pu = psA.tile([128, w], f32, tag="psA")
            for k in range(NKD):
                nc.tensor.matmul(pu[:], wu_s[:, k], xg_chunks[k // KC][:, k % KC, cs],
                                 start=(k == 0), stop=(k == NKD - 1))
            tmp = tmp_p.tile([128, 512], bf16, tag="tmp")
            nc.scalar.activation(tmp[:, :w], pg[:],
                                 mybir.ActivationFunctionType.Silu)
            nc.vector.tensor_mul(hT[:, h, cs], tmp[:, :w], pu[:])
    return hT


def _build_bass(Cs):
    """Per-core Tile program; Cs[j] = routed token capacity of expert slot j."""
    nc = bacc.Bacc(None, target_bir_lowering=False)

    xgt = [nc.dram_tensor(f"xgt{j}", [128, NKD, Cs[j]], bf16, kind="ExternalInput")
           for j in range(E_LOC)]
    wgu = nc.dram_tensor("wgu", [E_LOC, NH, 128, 2, NKD, 128], bf16, kind="ExternalInput")
    wdd = nc.dram_tensor("wdd", [E_LOC, 2 * ND, 128, NH, 2, 128], bf16, kind="ExternalInput")
    xtq = nc.dram_tensor("xtq", [NTQ, 128, NKD, TQ], bf16, kind="ExternalInput")
    swgu = nc.dram_tensor("swgu", [128, 2, NSH, NKD, 128], bf16, kind="ExternalInput")
    swdd = nc.dram_tensor("swdd", [128, ND, NSH, 512], bf16, kind="ExternalInput")
    y_out = [nc.dram_tensor(f"y_out{j}", [ND, 128, 4, Cs[j]], bf16, kind="ExternalOutput")
             for j in range(E_LOC)]
    ys_out = nc.dram_tensor("ys_out", [NTQ, ND, 128, TQ // 128, 512], bf16,
                            kind="ExternalOutput")

    with tile.TileContext(nc) as tc:
        with (
            tc.tile_pool(name="wgu_p", bufs=6) as wgu_p,
            tc.tile_pool(name="wd_p", bufs=8) as wd_p,
            tc.tile_pool(name="swgu_p", bufs=1) as swgu_p,
            tc.tile_pool(name="swd_p", bufs=1) as swd_p,
            tc.tile_pool(name="xg_p", bufs=2 * NXG) as xg_p,
            tc.tile_pool(name="xt_p", bufs=3) as xt_p,
            tc.tile_pool(name="ht_p", bufs=1) as ht_p,
            tc.tile_pool(name="hst_p", bufs=1) as hst_p,
            tc.tile_pool(name="tmp_p", bufs=2) as tmp_p,
            tc.tile_pool(name="y_p", bufs=4) as y_p,
            tc.tile_pool(name="psA", bufs=4, space="PSUM") as psA,
            tc.tile_pool(name="psB", bufs=4, space="PSUM") as psB,
        ):
            sg_slabs, sd_slabs, xq_tiles = [], [], [None] * NTQ

            # PE warm-up on zeros while the first loads land (HAM un-throttle);
            # short tail MMs so the real stream starts promptly when data lands
            warm = tmp_p.tile([128, 512], bf16, tag="tmp")
            nc.vector.memset(warm[:], 0.0)
            pwarm = psA.tile([128, 512], f32, tag="psA")
            for _ in range(8):
                nc.tensor.matmul(pwarm[:], warm[:, :128], warm[:], start=True, stop=True)
            for _ in range(4):
                nc.tensor.matmul(pwarm[:, :128], warm[:, :128], warm[:, :128],
                                 start=True, stop=True)

            # ---- expert-0 critical loads, interleaved in consumption order
            # so the first MM group streams as chunks land ----
            gu0 = wgu_p.tile([128, 2, NKD, 128], bf16, tag="wgu", name="wgu0_h0")
            xg0 = []

            # gate/up chunks on sync, xg chunks on scalar: the two issue
            # streams run in parallel so the first MM group starts sooner
            def _crit_gu(pr, ks):
                nc.sync.dma_start(gu0[:, pr, ks], wgu[0, 0, :, pr, ks])

            def _crit_xg(g):
                xc = xg_p.tile([128, KC, Cs[0]], bf16, tag="xg", name=f"xg0_{g}")
                nc.scalar.dma_start(xc[:], xgt[0][:, g * KC:(g + 1) * KC, :])
                xg0.append(xc)

            _crit_xg(0); _crit_xg(1); _crit_xg(2); _crit_xg(3)
            _crit_gu(0, slice(0, 4)); _crit_gu(0, slice(4, 8))
            _crit_gu(0, slice(8, 12)); _crit_gu(0, slice(12, 16))
            _crit_gu(1, slice(0, 8)); _crit_gu(1, slice(8, 16))
            slabs0 = [(gu0[:, 0], gu0[:, 1])]
            for h in range(1, NH):
                gu = wgu_p.tile([128, 2, NKD, 128], bf16, tag="wgu")
                nc.sync.dma_start(gu[:], wgu[0, h])
                slabs0.append((gu[:, 0], gu[:, 1]))
            # shared-expert weights + first two token quarters (resident)
            sgu = swgu_p.tile([128, 2, NSH, NKD, 128], bf16, tag="swgu")
            nc.sync.dma_start(sgu[:, 0], swgu[:, 0])
            nc.sync.dma_start(sgu[:, 1], swgu[:, 1])
            sg_slabs.append(sgu)
            sd = swd_p.tile([128, ND, NSH, 512], bf16, tag="swd")
            nc.sync.dma_start(sd[:], swdd[:])
            sd_slabs.append(sd)
            # token quarters load as (k 0-7 / k 8-15) half-tiles; 3 bufs
            # cycle {q-a, q-b, (q+1)-a} with JIT refill inside each quarter
            xq_tiles = [[None, None] for _ in range(NTQ)]

            def load_xq_half(q, half):
                t = xt_p.tile([128, NKD // 2, TQ], bf16, tag="xt",
                              name=f"xq{q}_{half}")
                nc.sync.dma_start(
                    t[:], xtq[q, :, half * (NKD // 2):(half + 1) * (NKD // 2)])
                xq_tiles[q][half] = t

            load_xq_half(0, 0)
            load_xq_half(0, 1)
            load_xq_half(1, 0)

            def load_xg(j):
                chunks = []
                for g in range(NXG):
                    xc = xg_p.tile([128, KC, Cs[j]], bf16, tag="xg", name=f"xg{j}_{g}")
                    nc.sync.dma_start(xc[:], xgt[j][:, g * KC:(g + 1) * KC, :])
                    chunks.append(xc)
                return chunks

            def load_wgu(j, h):
                gu = wgu_p.tile([128, 2, NKD, 128], bf16, tag="wgu", name=f"wgu{j}_h{h}")
                nc.sync.dma_start(gu[:], wgu[j, h])
                return (gu[:, 0], gu[:, 1])

            def shared_quarter(q):
                """One 512-token slice of the shared expert."""
                if q + 1 < NTQ and xq_tiles[q + 1][1] is None:
                    load_xq_half(q + 1, 1)
                if q + 2 < NTQ and xq_tiles[q + 2][0] is None:
                    load_xq_half(q + 2, 0)
                xq = xq_tiles[q]
                hsT = hst_p.tile([128, NSH, TQ], bf16, tag="hst")
                sgu = sg_slabs[0]
                for h in range(NSH):
                    pg = psA.tile([128, TQ], f32, tag="psA")
                    for k in range(NKD):
                        nc.tensor.matmul(pg[:], sgu[:, 0, h, k], xq[k // 8][:, k % 8],
                                         start=(k == 0), stop=(k == NKD - 1))
                    pu = psA.tile([128, TQ], f32, tag="psA")
                    for k in range(NKD):
                        nc.tensor.matmul(pu[:], sgu[:, 1, h, k], xq[k // 8][:, k % 8],
                                         start=(k == 0), stop=(k == NKD - 1))
                    tmp = tmp_p.tile([128, 512], bf16, tag="tmp")
                    nc.scalar.activation(tmp[:, :TQ], pg[:],
                                         mybir.ActivationFunctionType.Silu)
                    nc.vector.tensor_mul(hsT[:, h, :], tmp[:, :TQ], pu[:])
                for d in range(ND):
                    yst = y_p.tile([128, 4, 512], bf16, tag="y")
                    for ci in range(TQ // 128):
                        pp = psB if (d * 4 + ci) % 2 == 0 else psA
                        py = pp.tile([128, 512], f32, tag="psB" if pp is psB else "psA")
                        for h in range(NSH):
                            nc.tensor.matmul(py[:], hsT[:, h, ci * 128:(ci + 1) * 128],
                                             sd_slabs[0][:, d, h],
                                             start=(h == 0), stop=(h == NSH - 1))
                        nc.vector.tensor_copy(yst[:, ci], py[:])
                    nc.scalar.dma_start(ys_out[q, d], yst[:])

            xg_cur, slabs_cur = xg0, slabs0
            for j in range(E_LOC):
                C = Cs[j]
                hT = _expert_mlp(nc, (psA, tmp_p, ht_p),
                                 (xg_cur, slabs_cur), C, "ht", NH)
                # prefetch for the next expert's tokens, then ALL this
                # expert's down-proj weights (their buffer gates cleared in
                # B(j-1), so the whole 5.8MB streams during A(j)), then the
                # next expert's early weight slabs (gates spread over A(j))
                if j + 1 < E_LOC:
                    xg_cur = load_xg(j + 1)
                wd_slabs = []
                for dh in range(2 * ND):
                    wd_s = wd_p.tile([128, NH, 2, 128], bf16, tag="wd")
                    nc.sync.dma_start(wd_s[:], wdd[j, dh])
                    wd_slabs.append(wd_s)
                if j + 1 < E_LOC:
                    slabs_cur = [load_wgu(j + 1, h) for h in range(6)]
                # shared quarter: pure-compute DMA slack window
                shared_quarter(j)
                # phase B: stationary = wd d-tiles, moving = hT tokens;
                # 4 consecutive (dh,dt) outputs merge into one store
                NCC = -(-C // 512)
                assert NCC == 1, "C > 512 unsupported by merged stores"
                for qd in range(ND):
                    yst = y_p.tile([128, 4, C], bf16, tag="y")
                    split = (j == E_LOC - 1 and qd == ND - 1)
                    for g4 in range(4):
                        g = qd * 4 + g4
                        dh, dt = g // 2, g % 2
                        pp = psB if g % 2 == 0 else psA
                        py = pp.tile([128, 512], f32,
                                     tag="psB" if pp is psB else "psA")
                        for h in range(NH):
                            nc.tensor.matmul(py[:, :C], wd_slabs[dh][:, h, dt],
                                             hT[:, h, :],
                                             start=(h == 0), stop=(h == NH - 1))
                        nc.vector.tensor_copy(yst[:, g4], py[:, :C])
                        if split:
                            nc.scalar.dma_start(y_out[j][qd, :, g4], yst[:, g4])
                    if not split:
                        nc.scalar.dma_start(y_out[j][qd], yst[:])
                # late-gated slabs for the next expert (gates clear in A(j+1))
                if j + 1 < E_LOC:
                    slabs_cur = slabs_cur + [load_wgu(j + 1, h)
                                             for h in range(6, NH)]
    nc.compile()
    return nc


def kernel(hidden_states, gate_w, wg, wu, wd, swg, swu, swd):
    global LAST_RESULTS
    x = np.ascontiguousarray(np.asarray(hidden_states, np.float32).reshape(T, D))
    gate_w = np.asarray(gate_w, np.float32)
    wg = np.asarray(wg, np.float32)
    wu = np.asarray(wu, np.float32)
    wd = np.asarray(wd, np.float32)
    swg = np.asarray(swg, np.float32)
    swu = np.asarray(swu, np.float32)
    swd = np.asarray(swd, np.float32)

    # ---- host router ----
    topk_i, topk_w = _route(x, gate_w)
    idx = [np.where((topk_i == e).any(-1))[0] for e in range(E)]
    wts = [(topk_w * (topk_i == e))[idx[e]].sum(-1).astype(np.float32) for e in range(E)]
    cnts = np.array([len(i) for i in idx])
    # bucket experts: slot j on every core serves similarly-loaded experts
    ranked = np.argsort(-cnts, kind="stable")            # expert ids, busiest first
    emap = ranked.reshape(E_LOC, N_CORES)                # emap[j, c] -> expert id
    Cs = [max(16, -(-int(cnts[emap[j]].max()) // 4) * 4) for j in range(E_LOC)]

    nc = _build_bass(Cs)

    # ---- host shard + layout prep (all DMA sources partition-major) ----
    xT = np.ascontiguousarray(x.T)                      # [D, T] fp32
    xtq_np = np.ascontiguousarray(
        xT.reshape(NKD, 128, NTQ, TQ).transpose(2, 1, 0, 3).astype(BF16))

    in_maps = []
    for c in range(N_CORES):
        wgu_np = np.empty((E_LOC, NH, 128, 2, NKD, 128), BF16)
        wdd_np = np.empty((E_LOC, 2 * ND, 128, NH, 2, 128), BF16)
        imap = {"wgu": wgu_np, "wdd": wdd_np, "xtq": xtq_np}
        for j in range(E_LOC):
            e = int(emap[j, c])
            wgu_np[j] = (np.stack([wg[e], wu[e]])
                         .reshape(2, NKD, 128, NH, 128)
                         .transpose(3, 2, 0, 1, 4).astype(BF16))
            wdd_np[j] = (wd[e].reshape(NH, 128, 2 * ND, 2, 128)
                         .transpose(2, 1, 0, 3, 4).astype(BF16))
            cnt = int(cnts[e])
            xgt_np = np.zeros((128, NKD, Cs[j]), BF16)
            xg = xT[:, idx[e]]                          # [D, cnt] fp32
            xgt_np[:, :, :cnt] = (xg.reshape(NKD, 128, cnt)
                                  .transpose(1, 0, 2).astype(BF16))
            imap[f"xgt{j}"] = np.ascontiguousarray(xgt_np)
        sl = slice(c * HS_LOC, (c + 1) * HS_LOC)
        swg_c = np.zeros((D, HS_PAD), np.float32); swg_c[:, :HS_LOC] = swg[:, sl]
        swu_c = np.zeros((D, HS_PAD), np.float32); swu_c[:, :HS_LOC] = swu[:, sl]
        swd_c = np.zeros((HS_PAD, D), np.float32); swd_c[:HS_LOC] = swd[sl, :]
        imap["swgu"] = np.ascontiguousarray(
            np.stack([swg_c, swu_c]).reshape(2, NKD, 128, NSH, 128)
            .transpose(2, 0, 3, 1, 4).astype(BF16))
        imap["swdd"] = np.ascontiguousarray(
            swd_c.reshape(NSH, 128, ND, 512).transpose(1, 2, 0, 3).astype(BF16))
        in_maps.append(imap)

    res = run_bass_kernel_spmd(nc, in_maps, core_ids=list(range(N_CORES)))
    LAST_RESULTS = res

    # ---- host unshard: scatter-add routed outputs, sum shared partials ----
    out = np.zeros((T, D), np.float32)
    for c in range(N_CORES):
        ys = res.results[c]["ys_out"].astype(np.float32)   # [NTQ, ND, 128, 4, 512]
        out += ys.transpose(0, 3, 2, 1, 4).reshape(T, D)
        for j in range(E_LOC):
            e = int(emap[j, c])
            cnt = int(cnts[e])
            y = (res.results[c][f"y_out{j}"]               # [ND, 128, 4, Cs[j]] bf16
                 .transpose(0, 2, 1, 3).reshape(D, Cs[j])[:, :cnt].astype(np.float32))
            out[idx[e]] += (y * wts[e][None, :]).T
    return out.reshape(B, S, D)


# revision 24
# speedup vs baseline: 1.0100x; 1.0100x over previous
"""DeepseekOCR text MoE layer on 8 Trainium2 NeuronCores.

Expert-parallel: 4 routed experts per core (bucketed by token count so
every core's slot j has a similar load); shared expert tensor-sharded
over its intermediate dim (352 columns per core). Router + token
gather/scatter run on host (full-I/O contract); all matmuls run on
device in bf16 with fp32 PSUM accumulation; outputs stored bf16.

Device program per core, per expert cycle j:
  phase A (expert j):   hT[h,c] = silu(wg.T @ xgT) * (wu.T @ xgT)
  shared quarter q=j:   silu-mlp on a 512-token slice (this is the DMA
                        slack window for wd(j) + next expert's weights)
  phase B (expert j):   yT[d,c] = wd.T-tiles @ hT
Host: out = scatter_add(yT * combine_w) + sum_cores(ys).

Engine discipline: ALL loads issue on sync (emission ordered so a
semaphore-gated load never sits ahead of an earlier-needed one);
stores + silu on scalar (stores are gated just-in-time so they never
block a silu the PE pipeline needs). Per-slot dram tensors keep every
transfer contiguous per partition (fat descriptors).
"""

import numpy as np
import ml_dtypes

import concourse.bacc as bacc
import concourse.mybir as mybir
import concourse.tile as tile
from concourse.bass_utils import run_bass_kernel_spmd

B, S, D = 2, 1024, 2048
E, H, K = 32, 1408, 6
H_SHARED = 2816
ROUTED_SCALE = 1.0
T = B * S                      # 2048 tokens
N_CORES = 8
E_LOC = E // N_CORES           # 4 experts per core
HS_LOC = H_SHARED // N_CORES   # 352 shared-intermediate cols per core
HS_PAD = 384                   # padded to 3 k-tiles of 128
NH = H // 128                  # 11 h-tiles per routed expert
ND = D // 512                  # 4 d-groups (512 cols each)
NKD = D // 128                 # 16 contraction k-tiles over D
NSH = HS_PAD // 128            # 3 h-tiles for shared
TQ = 512                       # shared-expert token chunk
NTQ = T // TQ                  # 4 chunks
NXG = 4                        # xg split into k-chunks for early start
KC = NKD // NXG                # 4 k-tiles per xg chunk

BF16 = ml_dtypes.bfloat16
f32 = mybir.dt.float32
bf16 = mybir.dt.bfloat16

LAST_RESULTS = None            # BassKernelResults of the latest run (for test harness)


def _route(x, gate_w):
    """Greedy top-k softmax router, fp32 numpy (matches jax.lax.top_k order)."""
    logits = x @ gate_w.T                              # [T, E]
    m = logits.max(-1, keepdims=True)
    ex = np.exp(logits - m)
    scores = ex / ex.sum(-1, keepdims=True)
    topk_i = np.argsort(-scores, axis=-1, kind="stable")[:, :K]
    topk_w = np.take_along_axis(scores, topk_i, -1) * ROUTED_SCALE
    return topk_i, topk_w.astype(np.float32)


def _expert_mlp(nc, pools, slabs, C, hT_tag, nh):
    """Emit phase A (gate/up + silu*mul -> hT) for one expert."""
    psA, tmp_p, ht_p = pools
    xg_chunks, w_slabs = slabs                  # w_slabs[h] = (gate_slab, up_slab)
    NCC = -(-C // 512)
    hT = ht_p.tile([128, nh, C], bf16, tag=hT_tag)
    for h in range(nh):
        wg_s, wu_s = w_slabs[h]
        for cc in range(NCC):
            w = min(512, C - cc * 512)
            cs = slice(cc * 512, cc * 512 + w)
            pg = psA.tile([128, w], f32, tag="psA")
            for k in range(NKD):
                nc.tensor.matmul(pg[:], wg_s[:, k], xg_chunks[k // KC][:, k % KC, cs],
                                 start=(k == 0), stop=(k == NKD - 1))
            pu = psA.tile([128, w], f32, tag="psA")
            for k in range(NKD):
                nc.tensor.matmul(pu[:], wu_s[:, k], xg_chunks[k // KC][:, k % KC, cs],
                                 start=(k == 0), stop=(k == NKD - 1))
            tmp = tmp_p.tile([128, 512], bf16, tag="tmp")
            nc.scalar.activation(tmp[:, :w], pg[:],
                                 mybir.ActivationFunctionType.Silu)
            nc.vector.tensor_mul(hT[:, h, cs], tmp[:, :w], pu[:])
    return hT


def _build_bass(Cs):
    """Per-core Tile program; Cs[j] = routed token capacity of expert slot j."""
    nc = bacc.Bacc(None, target_bir_lowering=False)

    xgt = [nc.dram_tensor(f"xgt{j}", [128, NKD, Cs[j]], bf16, kind="ExternalInput")
           for j in range(E_LOC)]
    wgu = nc.dram_tensor("wgu", [E_LOC, NH, 128, 2, NKD, 128], bf16, kind="ExternalInput")
    wdd = nc.dram_tensor("wdd", [E_LOC, 2 * ND, 128, NH, 2, 128], bf16, kind="ExternalInput")
    xtq = nc.dram_tensor("xtq", [NTQ, 128, NKD, TQ], bf16, kind="ExternalInput")
    swgu = nc.dram_tensor("swgu", [128, 2, NSH, NKD, 128], bf16, kind="ExternalInput")
    swdd = nc.dram_tensor("swdd", [128, ND, NSH, 512], bf16, kind="ExternalInput")
    y_out = [nc.dram_tensor(f"y_out{j}", [ND, 128, 4, Cs[j]], bf16, kind="ExternalOutput")
             for j in range(E_LOC)]
    ys_out = nc.dram_tensor("ys_out", [NTQ, ND, 128, TQ // 128, 512], bf16,
                            kind="ExternalOutput")

    with tile.TileContext(nc) as tc:
        with (
            tc.tile_pool(name="wgu_p", bufs=6) as wgu_p,
            tc.tile_pool(name="wd_p", bufs=8) as wd_p,
            tc.tile_pool(name="swgu_p", bufs=1) as swgu_p,
            tc.tile_pool(name="swd_p", bufs=1) as swd_p,
            tc.tile_pool(name="xg_p", bufs=2 * NXG) as xg_p,
            tc.tile_pool(name="xt_p", bufs=3) as xt_p,
            tc.tile_pool(name="ht_p", bufs=1) as ht_p,
            tc.tile_pool(name="hst_p", bufs=1) as hst_p,
            tc.tile_pool(name="tmp_p", bufs=2) as tmp_p,
            tc.tile_pool(name="y_p", bufs=4) as y_p,
            tc.tile_pool(name="psA", bufs=4, space="PSUM") as psA,
            tc.tile_pool(name="psB", bufs=4, space="PSUM") as psB,
        ):
            sg_slabs, sd_slabs, xq_tiles = [], [], [None] * NTQ

            # PE warm-up on zeros while the first loads land (HAM un-throttle);
            # short tail MMs so the real stream starts promptly when data lands
            warm = tmp_p.tile([128, 512], bf16, tag="tmp")
            nc.vector.memset(warm[:], 0.0)
            pwarm = psA.tile([128, 512], f32, tag="psA")
            for _ in range(7):
                nc.tensor.matmul(pwarm[:], warm[:, :128], warm[:], start=True, stop=True)
            for _ in range(4):
                nc.tensor.matmul(pwarm[:, :128], warm[:, :128], warm[:, :128],
                                 start=True, stop=True)

            # ---- expert-0 critical loads, interleaved in consumption order
            # so the first MM group streams as chunks land ----
            gu0 = wgu_p.tile([128, 2, NKD, 128], bf16, tag="wgu", name="wgu0_h0")
            xg0 = []

            # interleaved in consumption order on ONE issue stream so the
            # first MM group's operands arrive in the order the k-loop needs
            def _crit_gu(pr, ks):
                nc.sync.dma_start(gu0[:, pr, ks], wgu[0, 0, :, pr, ks])

            def _crit_xg(g):
                xc = xg_p.tile([128, KC, Cs[0]], bf16, tag="xg", name=f"xg0_{g}")
                nc.sync.dma_start(xc[:], xgt[0][:, g * KC:(g + 1) * KC, :])
                xg0.append(xc)

            _crit_gu(0, slice(0, 4)); _crit_xg(0)
            _crit_gu(0, slice(4, 8)); _crit_xg(1)
            _crit_gu(0, slice(8, 12)); _crit_gu(1, slice(0, 8)); _crit_xg(2)
            _crit_gu(0, slice(12, 16)); _crit_xg(3)
            _crit_gu(1, slice(8, 16))
            slabs0 = [(gu0[:, 0], gu0[:, 1])]
            for h in range(1, NH):
                gu = wgu_p.tile([128, 2, NKD, 128], bf16, tag="wgu")
                nc.sync.dma_start(gu[:], wgu[0, h])
                slabs0.append((gu[:, 0], gu[:, 1]))
            # shared-expert weights + first two token quarters (resident)
            sgu = swgu_p.tile([128, 2, NSH, NKD, 128], bf16, tag="swgu")
            nc.sync.dma_start(sgu[:, 0], swgu[:, 0])
            nc.sync.dma_start(sgu[:, 1], swgu[:, 1])
            sg_slabs.append(sgu)
            sd = swd_p.tile([128, ND, NSH, 512], bf16, tag="swd")
            nc.sync.dma_start(sd[:], swdd[:])
            sd_slabs.append(sd)
            # token quarters load as (k 0-7 / k 8-15) half-tiles; 3 bufs
            # cycle {q-a, q-b, (q+1)-a} with JIT refill inside each quarter
            xq_tiles = [[None, None] for _ in range(NTQ)]

            def load_xq_half(q, half):
                t = xt_p.tile([128, NKD // 2, TQ], bf16, tag="xt",
                              name=f"xq{q}_{half}")
                nc.sync.dma_start(
                    t[:], xtq[q, :, half * (NKD // 2):(half + 1) * (NKD // 2)])
                xq_tiles[q][half] = t

            load_xq_half(0, 0)
            load_xq_half(0, 1)
            load_xq_half(1, 0)

            def load_xg(j):
                chunks = []
                for g in range(NXG):
                    xc = xg_p.tile([128, KC, Cs[j]], bf16, tag="xg", name=f"xg{j}_{g}")
                    nc.sync.dma_start(xc[:], xgt[j][:, g * KC:(g + 1) * KC, :])
                    chunks.append(xc)
                return chunks

            def load_wgu(j, h):
                gu = wgu_p.tile([128, 2, NKD, 128], bf16, tag="wgu", name=f"wgu{j}_h{h}")
                nc.sync.dma_start(gu[:], wgu[j, h])
                return (gu[:, 0], gu[:, 1])

            def shared_quarter(q):
                """One 512-token slice of the shared expert."""
                if q + 1 < NTQ and xq_tiles[q + 1][1] is None:
                    load_xq_half(q + 1, 1)
                if q + 2 < NTQ and xq_tiles[q + 2][0] is None:
                    load_xq_half(q + 2, 0)
                xq = xq_tiles[q]
                hsT = hst_p.tile([128, NSH, TQ], bf16, tag="hst")
                sgu = sg_slabs[0]
                for h in range(NSH):
                    pg = psA.tile([128, TQ], f32, tag="psA")
                    for k in range(NKD):
                        nc.tensor.matmul(pg[:], sgu[:, 0, h, k], xq[k // 8][:, k % 8],
                                         start=(k == 0), stop=(k == NKD - 1))
                    pu = psA.tile([128, TQ], f32, tag="psA")
                    for k in range(NKD):
                        nc.tensor.matmul(pu[:], sgu[:, 1, h, k], xq[k // 8][:, k % 8],
                                         start=(k == 0), stop=(k == NKD - 1))
                    tmp = tmp_p.tile([128, 512], bf16, tag="tmp")
                    nc.scalar.activation(tmp[:, :TQ], pg[:],
                                         mybir.ActivationFunctionType.Silu)
                    nc.vector.tensor_mul(hsT[:, h, :], tmp[:, :TQ], pu[:])
                for d in range(ND):
                    yst = y_p.tile([128, 4, 512], bf16, tag="y")
                    for ci in range(TQ // 128):
                        pp = psB if (d * 4 + ci) % 2 == 0 else psA
                        py = pp.tile([128, 512], f32, tag="psB" if pp is psB else "psA")
                        for h in range(NSH):
                            nc.tensor.matmul(py[:], hsT[:, h, ci * 128:(ci + 1) * 128],
                                             sd_slabs[0][:, d, h],
                                             start=(h == 0), stop=(h == NSH - 1))
                        nc.vector.tensor_copy(yst[:, ci], py[:])
                    nc.scalar.dma_start(ys_out[q, d], yst[:])

            xg_cur, slabs_cur = xg0, slabs0
            for j in range(E_LOC):
                C = Cs[j]
                hT = _expert_mlp(nc, (psA, tmp_p, ht_p),
                                 (xg_cur, slabs_cur), C, "ht", NH)
                # prefetch for the next expert's tokens, then ALL this
                # expert's down-proj weights (their buffer gates cleared in
                # B(j-1), so the whole 5.8MB streams during A(j)), then the
                # next expert's early weight slabs (gates spread over A(j))
                if j + 1 < E_LOC:
                    xg_cur = load_xg(j + 1)
                wd_slabs = []
                for dh in range(2 * ND):
                    wd_s = wd_p.tile([128, NH, 2, 128], bf16, tag="wd")
                    nc.sync.dma_start(wd_s[:], wdd[j, dh])
                    wd_slabs.append(wd_s)
                if j + 1 < E_LOC:
                    slabs_cur = [load_wgu(j + 1, h) for h in range(6)]
                # shared quarter: pure-compute DMA slack window
                shared_quarter(j)
                # phase B: stationary = wd d-tiles, moving = hT tokens;
                # 4 consecutive (dh,dt) outputs merge into one store
                NCC = -(-C // 512)
                assert NCC == 1, "C > 512 unsupported by merged stores"
                for qd in range(ND):
                    yst = y_p.tile([128, 4, C], bf16, tag="y")
                    split = (j == E_LOC - 1 and qd == ND - 1)
                    for g4 in range(4):
                        g = qd * 4 + g4
                        dh, dt = g // 2, g % 2
                        pp = psB if g % 2 == 0 else psA
                        py = pp.tile([128, 512], f32,
                                     tag="psB" if pp is psB else "psA")
                        for h in range(NH):
                            nc.tensor.matmul(py[:, :C], wd_slabs[dh][:, h, dt],
                                             hT[:, h, :],
                                             start=(h == 0), stop=(h == NH - 1))
                        nc.vector.tensor_copy(yst[:, g4], py[:, :C])
                        if split:
                            nc.scalar.dma_start(y_out[j][qd, :, g4], yst[:, g4])
                    if not split:
                        nc.scalar.dma_start(y_out[j][qd], yst[:])
                # late-gated slabs for the next expert (gates clear in A(j+1))
                if j + 1 < E_LOC:
                    slabs_cur = slabs_cur + [load_wgu(j + 1, h)
                                             for h in range(6, NH)]
    nc.compile()
    return nc


def kernel(hidden_states, gate_w, wg, wu, wd, swg, swu, swd):
    global LAST_RESULTS
    x = np.ascontiguousarray(np.asarray(hidden_states, np.float32).reshape(T, D))
    gate_w = np.asarray(gate_w, np.float32)
    wg = np.asarray(wg, np.float32)
    wu = np.asarray(wu, np.float32)
    wd = np.asarray(wd, np.float32)
    swg = np.asarray(swg, np.float32)
    swu = np.asarray(swu, np.float32)
    swd = np.asarray(swd, np.float32)

    # ---- host router ----
    topk_i, topk_w = _route(x, gate_w)
    idx = [np.where((topk_i == e).any(-1))[0] for e in range(E)]
    wts = [(topk_w * (topk_i == e))[idx[e]].sum(-1).astype(np.float32) for e in range(E)]
    cnts = np.array([len(i) for i in idx])
    # bucket experts: slot j on every core serves similarly-loaded experts
    ranked = np.argsort(-cnts, kind="stable")            # expert ids, busiest first
    emap = ranked.reshape(E_LOC, N_CORES)                # emap[j, c] -> expert id
    Cs = [max(16, -(-int(cnts[emap[j]].max()) // 4) * 4) for j in range(E_LOC)]

    nc = _build_bass(Cs)

    # ---- host shard + layout prep (all DMA sources partition-major) ----
    xT = np.ascontiguousarray(x.T)                      # [D, T] fp32
    xtq_np = np.ascontiguousarray(
        xT.reshape(NKD, 128, NTQ, TQ).transpose(2, 1, 0, 3).astype(BF16))

    in_maps = []
    for c in range(N_CORES):
        wgu_np = np.empty((E_LOC, NH, 128, 2, NKD, 128), BF16)
        wdd_np = np.empty((E_LOC, 2 * ND, 128, NH, 2, 128), BF16)
        imap = {"wgu": wgu_np, "wdd": wdd_np, "xtq": xtq_np}
        for j in range(E_LOC):
            e = int(emap[j, c])
            wgu_np[j] = (np.stack([wg[e], wu[e]])
                         .reshape(2, NKD, 128, NH, 128)
                         .transpose(3, 2, 0, 1, 4).astype(BF16))
            wdd_np[j] = (wd[e].reshape(NH, 128, 2 * ND, 2, 128)
                         .transpose(2, 1, 0, 3, 4).astype(BF16))
            cnt = int(cnts[e])
            xgt_np = np.zeros((128, NKD, Cs[j]), BF16)
            xg = xT[:, idx[e]]                          # [D, cnt] fp32
            xgt_np[:, :, :cnt] = (xg.reshape(NKD, 128, cnt)
                                  .transpose(1, 0, 2).astype(BF16))
            imap[f"xgt{j}"] = np.ascontiguousarray(xgt_np)
        sl = slice(c * HS_LOC, (c + 1) * HS_LOC)
        swg_c = np.zeros((D, HS_PAD), np.float32); swg_c[:, :HS_LOC] = swg[:, sl]
        swu_c = np.zeros((D, HS_PAD), np.float32); swu_c[:, :HS_LOC] = swu[:, sl]
        swd_c = np.zeros((HS_PAD, D), np.float32); swd_c[:HS_LOC] = swd[sl, :]
        imap["swgu"] = np.ascontiguousarray(
            np.stack([swg_c, swu_c]).reshape(2, NKD, 128, NSH, 128)
            .transpose(2, 0, 3, 1, 4).astype(BF16))
        imap["swdd"] = np.ascontiguousarray(
            swd_c.reshape(NSH, 128, ND, 512).transpose(1, 2, 0, 3).astype(BF16))
        in_maps.append(imap)

    res = run_bass_kernel_spmd(nc, in_maps, core_ids=list(range(N_CORES)))
    LAST_RESULTS = res

    # ---- host unshard: scatter-add routed outputs, sum shared partials ----
    out = np.zeros((T, D), np.float32)
    for c in range(N_CORES):
        ys = res.results[c]["ys_out"].astype(np.float32)   # [NTQ, ND, 128, 4, 512]
        out += ys.transpose(0, 3, 2, 1, 4).reshape(T, D)
        for j in range(E_LOC):
            e = int(emap[j, c])
            cnt = int(cnts[e])
            y = (res.results[c][f"y_out{j}"]               # [ND, 128, 4, Cs[j]] bf16
                 .transpose(0, 2, 1, 3).reshape(D, Cs[j])[:, :cnt].astype(np.float32))
            out[idx[e]] += (y * wts[e][None, :]).T
    return out.reshape(B, S, D)


# revision 25
# speedup vs baseline: 1.0329x; 1.0226x over previous
"""DeepseekOCR text MoE layer on 8 Trainium2 NeuronCores.

Expert-parallel with a unified slot structure: each core runs FIVE
identical expert cycles — 4 routed experts (bucketed by token count so
every core's slot has a similar load) plus the shared expert as a 5th
slot. The shared expert is 2D-sharded (4 token-quarters x 2 halves of
its 2816-wide intermediate): each core computes one (quarter, half)
cell = 512 tokens x 1408 cols = exactly 11 partition tiles, so the
352-column padding waste of 1D sharding disappears and the shared
weights STREAM through the same pools as expert weights (no residency).

Device program per core, per slot s (C = slot token capacity):
  phase A:  hT[h,c] = silu(wg.T @ xT) * (wu.T @ xT)   (16 k-tile acc)
  phase B:  yT[d,c] = wd.T-tiles @ hT                 (11 h-tile acc)
Host: routed slots scatter-add yT * combine_w; shared slot adds yT.T
into its token quarter (2 cores per quarter, halves sum).

Engine discipline: all loads issue on sync, emission-ordered so a
semaphore-gated load never sits ahead of an earlier-needed one; stores
+ silu on scalar, merged 4 output tiles per store so trailing stores
never delay a silu. B-phase PSUM groups alternate across both pools
(8 banks of elasticity against store-completion lag).
"""

import numpy as np
import ml_dtypes

import concourse.bacc as bacc
import concourse.mybir as mybir
import concourse.tile as tile
from concourse.bass_utils import run_bass_kernel_spmd

B, S, D = 2, 1024, 2048
E, H, K = 32, 1408, 6
H_SHARED = 2816
ROUTED_SCALE = 1.0
T = B * S                      # 2048 tokens
N_CORES = 8
E_LOC = E // N_CORES           # 4 routed experts per core
NSLOT = E_LOC + 1              # + shared-expert slot
SH_POS = 1                     # shared slot position in the cycle order
HS_LOC = H_SHARED // 2         # 1408 shared cols per core (2-way split)
TQ = 512                       # shared-expert token quarter
NH = H // 128                  # 11 h-tiles per slot (routed == shared half)
ND = D // 512                  # 4 d-groups (512 cols each)
NKD = D // 128                 # 16 contraction k-tiles over D
NXG = 4                        # x split into k-chunks for early start
KC = NKD // NXG                # 4 k-tiles per chunk

BF16 = ml_dtypes.bfloat16
f32 = mybir.dt.float32
bf16 = mybir.dt.bfloat16

LAST_RESULTS = None            # BassKernelResults of the latest run (for test harness)


def _route(x, gate_w):
    """Greedy top-k softmax router, fp32 numpy (matches jax.lax.top_k order)."""
    logits = x @ gate_w.T                              # [T, E]
    m = logits.max(-1, keepdims=True)
    ex = np.exp(logits - m)
    scores = ex / ex.sum(-1, keepdims=True)
    topk_i = np.argsort(-scores, axis=-1, kind="stable")[:, :K]
    topk_w = np.take_along_axis(scores, topk_i, -1) * ROUTED_SCALE
    return topk_i, topk_w.astype(np.float32)


def _build_bass(Cs):
    """Per-core Tile program; Cs[s] = token capacity of slot s (len NSLOT)."""
    nc = bacc.Bacc(None, target_bir_lowering=False)

    xgt = [nc.dram_tensor(f"xgt{s}", [128, NKD, Cs[s]], bf16, kind="ExternalInput")
           for s in range(NSLOT)]
    wgu = nc.dram_tensor("wgu", [NSLOT, NH, 128, 2, NKD, 128], bf16,
                         kind="ExternalInput")
    wdd = nc.dram_tensor("wdd", [NSLOT, 2 * ND, 128, NH, 2, 128], bf16,
                         kind="ExternalInput")
    y_out = [nc.dram_tensor(f"y_out{s}", [ND, 128, 4, Cs[s]], bf16,
                            kind="ExternalOutput")
             for s in range(NSLOT)]

    with tile.TileContext(nc) as tc:
        with (
            tc.tile_pool(name="wgu_p", bufs=10) as wgu_p,
            tc.tile_pool(name="wd_p", bufs=8) as wd_p,
            tc.tile_pool(name="xg_p", bufs=2 * NXG) as xg_p,
            tc.tile_pool(name="ht_p", bufs=1) as ht_p,
            tc.tile_pool(name="tmp_p", bufs=2) as tmp_p,
            tc.tile_pool(name="y_p", bufs=6) as y_p,
            tc.tile_pool(name="psA", bufs=4, space="PSUM") as psA,
            tc.tile_pool(name="psB", bufs=4, space="PSUM") as psB,
        ):
            # PE warm-up on zeros while the first loads land (HAM un-throttle);
            # short tail MMs so the real stream starts promptly when data lands
            warm = tmp_p.tile([128, 512], bf16, tag="tmp")
            nc.vector.memset(warm[:], 0.0)
            pwarm = psA.tile([128, 512], f32, tag="psA")
            for _ in range(7):
                nc.tensor.matmul(pwarm[:], warm[:, :128], warm[:], start=True, stop=True)
            for _ in range(4):
                nc.tensor.matmul(pwarm[:, :128], warm[:, :128], warm[:, :128],
                                 start=True, stop=True)

            # ---- slot-0 critical loads, interleaved in consumption order on
            # ONE issue stream so the first MM group's operands arrive in the
            # order the k-loop needs ----
            gu0 = wgu_p.tile([128, 2, NKD, 128], bf16, tag="wgu", name="wgu0_h0")
            xg0 = []

            def _crit_gu(pr, ks):
                nc.sync.dma_start(gu0[:, pr, ks], wgu[0, 0, :, pr, ks])

            def _crit_xg(g):
                xc = xg_p.tile([128, KC, Cs[0]], bf16, tag="xg", name=f"xg0_{g}")
                nc.sync.dma_start(xc[:], xgt[0][:, g * KC:(g + 1) * KC, :])
                xg0.append(xc)

            _crit_gu(0, slice(0, 4)); _crit_xg(0)
            _crit_gu(0, slice(4, 8)); _crit_xg(1)
            _crit_gu(0, slice(8, 12)); _crit_gu(1, slice(0, 8)); _crit_xg(2)
            _crit_gu(0, slice(12, 16)); _crit_xg(3)
            _crit_gu(1, slice(8, 16))
            slabs0 = [(gu0[:, 0], gu0[:, 1])]
            for h in range(1, NH):
                gu = wgu_p.tile([128, 2, NKD, 128], bf16, tag="wgu")
                nc.sync.dma_start(gu[:], wgu[0, h])
                slabs0.append((gu[:, 0], gu[:, 1]))

            def load_xg(s):
                chunks = []
                for g in range(NXG):
                    xc = xg_p.tile([128, KC, Cs[s]], bf16, tag="xg", name=f"xg{s}_{g}")
                    nc.sync.dma_start(xc[:], xgt[s][:, g * KC:(g + 1) * KC, :])
                    chunks.append(xc)
                return chunks

            def load_wgu(s, h):
                gu = wgu_p.tile([128, 2, NKD, 128], bf16, tag="wgu", name=f"wgu{s}_h{h}")
                nc.sync.dma_start(gu[:], wgu[s, h])
                return (gu[:, 0], gu[:, 1])

            xg_cur, slabs_cur = xg0, slabs0
            for s in range(NSLOT):
                C = Cs[s]
                # ---- phase A: gate/up projections + silu*mul -> hT ----
                hT = ht_p.tile([128, NH, C], bf16, tag="ht")
                for h in range(NH):
                    wg_s, wu_s = slabs_cur[h]
                    pg = psA.tile([128, C], f32, tag="psA")
                    for k in range(NKD):
                        nc.tensor.matmul(pg[:], wg_s[:, k], xg_cur[k // KC][:, k % KC],
                                         start=(k == 0), stop=(k == NKD - 1))
                    pu = psA.tile([128, C], f32, tag="psA")
                    for k in range(NKD):
                        nc.tensor.matmul(pu[:], wu_s[:, k], xg_cur[k // KC][:, k % KC],
                                         start=(k == 0), stop=(k == NKD - 1))
                    tmp = tmp_p.tile([128, 512], bf16, tag="tmp")
                    nc.scalar.activation(tmp[:, :C], pg[:],
                                         mybir.ActivationFunctionType.Silu)
                    nc.vector.tensor_mul(hT[:, h, :], tmp[:, :C], pu[:])

                # next slot's tokens, THIS slot's down-proj weights (ungated:
                # buffers freed in B(s-1), so the stream runs during A(s)),
                # then the next slot's weight slabs (gates spread over A(s))
                if s + 1 < NSLOT:
                    xg_cur = load_xg(s + 1)
                wd_slabs = []
                for dh in range(2 * ND):
                    wd_s = wd_p.tile([128, NH, 2, 128], bf16, tag="wd")
                    nc.sync.dma_start(wd_s[:], wdd[s, dh])
                    wd_slabs.append(wd_s)
                if s + 1 < NSLOT:
                    slabs_cur = [load_wgu(s + 1, h) for h in range(9)]

                # ---- phase B: stationary = wd d-tiles, moving = hT tokens;
                # 4 consecutive (dh,dt) outputs merge into one store ----
                for qd in range(ND):
                    yst = y_p.tile([128, 4, C], bf16, tag="y")
                    split = (s == NSLOT - 1 and qd == ND - 1)
                    for g4 in range(4):
                        g = qd * 4 + g4
                        dh, dt = g // 2, g % 2
                        pp = psB if g % 2 == 0 else psA
                        py = pp.tile([128, 512], f32,
                                     tag="psB" if pp is psB else "psA")
                        for h in range(NH):
                            nc.tensor.matmul(py[:, :C], wd_slabs[dh][:, h, dt],
                                             hT[:, h, :],
                                             start=(h == 0), stop=(h == NH - 1))
                        nc.vector.tensor_copy(yst[:, g4], py[:, :C])
                        if split:
                            nc.scalar.dma_start(y_out[s][qd, :, g4], yst[:, g4])
                    if not split:
                        nc.scalar.dma_start(y_out[s][qd], yst[:])
                # late slabs for the next slot (gates clear in A(s+1))
                if s + 1 < NSLOT:
                    slabs_cur = slabs_cur + [load_wgu(s + 1, h)
                                             for h in range(9, NH)]
    nc.compile()
    return nc


def _wgu_pack(g, u):
    """[D,Hc] gate/up -> [NH, 128, 2, NKD, 128] slab layout."""
    return (np.stack([g, u]).reshape(2, NKD, 128, NH, 128)
            .transpose(3, 2, 0, 1, 4).astype(BF16))


def _wdd_pack(w):
    """[Hc,D] down -> [2*ND, 128, NH, 2, 128] slab layout."""
    return (w.reshape(NH, 128, 2 * ND, 2, 128)
            .transpose(2, 1, 0, 3, 4).astype(BF16))


def _xgt_pack(xg, C):
    """[D,cnt] gathered tokens -> [128, NKD, C] chunk layout."""
    cnt = xg.shape[1]
    out = np.zeros((128, NKD, C), BF16)
    out[:, :, :cnt] = xg.reshape(NKD, 128, cnt).transpose(1, 0, 2).astype(BF16)
    return np.ascontiguousarray(out)


def kernel(hidden_states, gate_w, wg, wu, wd, swg, swu, swd):
    global LAST_RESULTS
    x = np.ascontiguousarray(np.asarray(hidden_states, np.float32).reshape(T, D))
    gate_w = np.asarray(gate_w, np.float32)
    wg = np.asarray(wg, np.float32)
    wu = np.asarray(wu, np.float32)
    wd = np.asarray(wd, np.float32)
    swg = np.asarray(swg, np.float32)
    swu = np.asarray(swu, np.float32)
    swd = np.asarray(swd, np.float32)

    # ---- host router ----
    topk_i, topk_w = _route(x, gate_w)
    idx = [np.where((topk_i == e).any(-1))[0] for e in range(E)]
    wts = [(topk_w * (topk_i == e))[idx[e]].sum(-1).astype(np.float32) for e in range(E)]
    cnts = np.array([len(i) for i in idx])
    # bucket experts: slot j on every core serves similarly-loaded experts
    ranked = np.argsort(-cnts, kind="stable")            # expert ids, busiest first
    emap = ranked.reshape(E_LOC, N_CORES)                # emap[j, c] -> expert id
    Cr = [max(16, -(-int(cnts[emap[j]].max()) // 4) * 4) for j in range(E_LOC)]
    # cycle order: routed0, shared, routed1..3 (shared slot at SH_POS)
    Cs = Cr[:SH_POS] + [TQ] + Cr[SH_POS:]
    jmap = list(range(SH_POS)) + [None] + list(range(SH_POS, E_LOC))

    nc = _build_bass(Cs)

    # ---- host shard + layout prep (all DMA sources partition-major) ----
    xT = np.ascontiguousarray(x.T)                      # [D, T] fp32

    in_maps = []
    for c in range(N_CORES):
        qc, hc = c >> 1, c & 1                          # shared (quarter, half)
        wgu_np = np.empty((NSLOT, NH, 128, 2, NKD, 128), BF16)
        wdd_np = np.empty((NSLOT, 2 * ND, 128, NH, 2, 128), BF16)
        imap = {"wgu": wgu_np, "wdd": wdd_np}
        hs = slice(hc * HS_LOC, (hc + 1) * HS_LOC)
        for s in range(NSLOT):
            j = jmap[s]
            if j is None:                               # shared-expert slot
                wgu_np[s] = _wgu_pack(swg[:, hs], swu[:, hs])
                wdd_np[s] = _wdd_pack(swd[hs, :])
                imap[f"xgt{s}"] = _xgt_pack(xT[:, qc * TQ:(qc + 1) * TQ], TQ)
            else:
                e = int(emap[j, c])
                wgu_np[s] = _wgu_pack(wg[e], wu[e])
                wdd_np[s] = _wdd_pack(wd[e])
                imap[f"xgt{s}"] = _xgt_pack(xT[:, idx[e]], Cs[s])
        in_maps.append(imap)

    res = run_bass_kernel_spmd(nc, in_maps, core_ids=list(range(N_CORES)))
    LAST_RESULTS = res

    # ---- host unshard: scatter-add routed outputs, add shared partials ----
    out = np.zeros((T, D), np.float32)
    for c in range(N_CORES):
        qc = c >> 1
        for s in range(NSLOT):
            y = (res.results[c][f"y_out{s}"]            # [ND, 128, 4, Cs[s]] bf16
                 .transpose(0, 2, 1, 3).reshape(D, Cs[s]).astype(np.float32))
            j = jmap[s]
            if j is None:
                out[qc * TQ:(qc + 1) * TQ] += y.T
            else:
                e = int(emap[j, c])
                cnt = int(cnts[e])
                out[idx[e]] += (y[:, :cnt] * wts[e][None, :]).T
    return out.reshape(B, S, D)
